# revision 30
# baseline (speedup 1.0000x reference)
"""Single-head causal attention (B=8, T=2048, E=1024, H=64) on 8 TRN2 cores.

Sharding: data-parallel over batch B - one batch element per NeuronCore;
projection weights replicated. Per-core kernel:

  q = x @ Wq.T + bq ; k = x @ Wk.T + bk ; v = x @ Wv.T + bv
  s = (q @ k.T) * sqrt(H)  (scale folded into Wq/bq on host)
  causal softmax(s) @ v

Design (v2) - all-matmul PE stream, no PE transposes (keeps HAM warm):
  - x pre-transposed + cast fp16 on host -> contiguous DMA (no xbar).
  - Q^T/K^T packed projection (full 128-wide array), written into
    65-partition tiles qa/ka; ka row 64 = ones, qa row 64 = -m_i (per
    q-block row max, folded into the scores via the augmented
    contraction: S^T = ka.T-slice @ qa-slice includes -m per column).
  - Max pass: S = q-block @ k^T chunks in PSUM, row max via fused
    tensor_tensor_reduce (2 elem/cycle, negation folded via scale=-1 +
    min-reduce), chained across chunks -> -m in fp16.
  - -m row vector obtained with a tiny matmul against identity
    (negm^T = negm.T @ I) - no transpose-mode ops anywhere.
  - S^T computed directly by a second matmul (lhsT = ka j-block,
    rhs = qa pair-block), exp on ACT reads PSUM -> writes P^T (bf16)
    straight to SBUF. Diagonal/causal masking via one fp16 multiply.
  - q-blocks processed in pairs (256-wide moving operand).
  - AV in O^T form: lhsT = v-tile [128,65] (col 64 = ones -> row sum l),
    rhs = P^T pair tile, accumulate O^T [65, 256] in PSUM. Normalize by
    broadcasting 1/l (DMA broadcast) + one multiply; output stored as
    O^T [64, T] and un-transposed on host.
"""

import sys

sys.path.insert(0, "/opt/trn_rl_repo")

import numpy as np

import concourse.bass as bass
import concourse.mybir as mybir
from concourse import bacc
from concourse.bass import ds, ts
from concourse.tile import TileContext

B, T, E, H = 8, 2048, 1024, 64
P = 128
NE = E // P  # 8 e-chunks
NT = T // P  # 16 t-tiles
NPAIR = NT // 2  # 8 q-block pairs
F16 = mybir.dt.float16
BF16 = mybir.dt.bfloat16
F32 = mybir.dt.float32

_CACHE = {}
DEBUG = False


def build_nc():
    nc = bacc.Bacc("TRN2", num_devices=8)
    x16t = nc.declare_dram_parameter("x16t", [E, T], F16, isOutput=False)
    wqkT = nc.declare_dram_parameter("wqkT", [E, P], F16, isOutput=False)
    wvT = nc.declare_dram_parameter("wvT", [E, H], F16, isOutput=False)
    bqk = nc.declare_dram_parameter("bqk", [P, 1], F32, isOutput=False)
    bv4 = nc.declare_dram_parameter("bv4", [1, 4 * H], F32, isOutput=False)
    trqk = nc.declare_dram_parameter("trqk", [P, P], F32, isOutput=False)
    trkq = nc.declare_dram_parameter("trkq", [P, P], F16, isOutput=False)
    id128 = nc.declare_dram_parameter("id128", [P, P], F16, isOutput=False)
    out = nc.declare_dram_parameter("out", [T, H], F32, isOutput=True)
    if DEBUG:
        qa_d = nc.declare_dram_parameter("qa_d", [H + 1, T], F16, isOutput=True)
        ka_d = nc.declare_dram_parameter("ka_d", [H + 1, T], F16, isOutput=True)
        vt_d = nc.declare_dram_parameter(
            "vt_d", [P, NT * (H + 1)], F16, isOutput=True
        )

    with TileContext(nc) as tc:
        with (
            tc.tile_pool(name="const", bufs=1) as cpool,
            tc.tile_pool(name="xt", bufs=1) as xtpool,
            tc.tile_pool(name="qk", bufs=1) as qkpool,
            tc.tile_pool(name="vp", bufs=1) as vpool,
            tc.tile_pool(name="pt", bufs=18) as ptpool,
            tc.tile_pool(name="negm", bufs=2) as negmpool,
            tc.tile_pool(name="rl", bufs=2) as rlpool,
            tc.tile_pool(name="osb", bufs=2) as opool,
        ):
            # ---- constants ----
            wqk_sb = cpool.tile([P, NE, P], F16)
            nc.sync.dma_start(
                out=wqk_sb, in_=wqkT.rearrange("(c p) h -> p c h", p=P)
            )
            wv_sb = cpool.tile([P, NE, H], F16)
            nc.sync.dma_start(out=wv_sb, in_=wvT.rearrange("(c p) h -> p c h", p=P))
            bqk_sb = cpool.tile([P, 1], F32)
            nc.sync.dma_start(out=bqk_sb, in_=bqk[:, :])
            bv_sb = cpool.tile([P, 4 * H], F32)
            nc.sync.dma_start(out=bv_sb, in_=bv4[:, :].to_broadcast((P, 4 * H)))
            trqk_sb = cpool.tile([P, P], F32)
            nc.sync.dma_start(out=trqk_sb, in_=trqk[:, :])
            trkq_sb = cpool.tile([P, P], F16)
            nc.sync.dma_start(out=trkq_sb, in_=trkq[:, :])
            id_sb = cpool.tile([P, P], F16)
            nc.sync.dma_start(out=id_sb, in_=id128[:, :])

            # ---- x^T chunks (contiguous, pre-transposed on host) ----
            xt = xtpool.tile([P, NE, T], F16)
            for c in range(NE):
                nc.sync.dma_start(out=xt[:, c, :], in_=x16t[ts(c, P), :])

            # qa/ka: rows 0..63 = q^T/k^T; row 64: ka = ones, qa = -m
            qa = qkpool.tile([H + 1, T], F16)
            ka = qkpool.tile([H + 1, T], F16)
            nc.vector.memset(ka[H : H + 1, :], 1.0)

            # vt: [128, NT, H+1]; col H = ones (row-sum trick)
            vt = vpool.tile([P, NT, H + 1], F16)
            nc.vector.memset(vt, 1.0)
            neginf = cpool.tile([P, 1024], F32)
            nc.vector.memset(neginf, -3.0e38)

            # ---- phase 1: projections (DMA-overlapped c-major loop) ----
            with (
                tc.tile_pool(name="accq", bufs=1, space="PSUM") as accqp,
                tc.tile_pool(name="accv", bufs=1, space="PSUM") as accvp,
            ):
                acc0 = accqp.tile([P, 1024], F32, tag="a0")
                acc1 = accqp.tile([P, 1024], F32, tag="a1")
                vacc = accvp.tile([P, NT * H], F32, tag="v")
                accs = [acc0, acc1]
                for c in range(NE):
                    for n in range(2):
                        for h in range(2):
                            nc.tensor.matmul(
                                accs[n][:, ds(h * 512, 512)],
                                lhsT=wqk_sb[:, c, :],
                                rhs=xt[:, c, ds(n * 1024 + h * 512, 512)],
                                start=(c == 0),
                                stop=(c == NE - 1),
                            )
                    for t in range(NT):
                        # start=True clears the whole PSUM bank's has_written
                        # bits, so only the first t-group per bank may set it;
                        # later groups overwrite via has_written=0.
                        nc.tensor.matmul(
                            vacc[:, ds(t * H, H)],
                            lhsT=xt[:, c, ts(t, P)],
                            rhs=wv_sb[:, c, :],
                            start=(c == 0 and t % 8 == 0),
                            stop=(c == NE - 1),
                            skip_group_check=True,
                        )
                # write q^T/k^T (+bias) into qa/ka rows 0..63
                for n in range(2):
                    nc.scalar.activation(
                        out=qa[0:H, ds(n * 1024, 1024)],
                        in_=accs[n][0:H, :],
                        func=mybir.ActivationFunctionType.Identity,
                        bias=bqk_sb[0:H, :],
                        scale=1.0,
                    )
                    nc.scalar.activation(
                        out=ka[0:H, ds(n * 1024, 1024)],
                        in_=accs[n][H:P, :],
                        func=mybir.ActivationFunctionType.Identity,
                        bias=bqk_sb[H:P, :],
                        scale=1.0,
                    )
                # v + bias -> vt cols 0..63 (quads of t-tiles)
                for g in range(4):
                    nc.vector.tensor_add(
                        vt[:, ds(g * 4, 4), 0:H],
                        vacc[:, ds(g * 4 * H, 4 * H)].rearrange(
                            "p (t h) -> p t h", t=4
                        ),
                        bv_sb[:, :].rearrange("p (t h) -> p t h", t=4),
                    )

            # ---- phase 2: attention over q-block pairs ----
            with (
                tc.tile_pool(name="sps", bufs=2, space="PSUM") as spool,
                tc.tile_pool(name="stp", bufs=2, space="PSUM") as stpool,
                tc.tile_pool(name="av", bufs=1, space="PSUM") as avpool,
                tc.tile_pool(name="rowm", bufs=1, space="PSUM") as rmpool,
            ):
                scr = cpool.tile([P, 1024], F32)  # TT-reduce throwaway out
                state = {}

                def spass_thunks(r):
                    """Max pass for pair r: S chunks + chained TT-reduce
                    (negation via scale=-1 + min; causal diag via op0=add
                    with the -1e9 mask; exact causal row max)."""
                    negm = negmpool.tile([P, 8], F16)
                    thunks = []
                    steps = [0, 0]
                    for half in range(2):
                        i = 2 * r + half
                        w = (i + 1) * P
                        chain = []  # (in0_slice_fn, in1, op0, width)
                        for t0 in range(0, w, 1024):
                            tw = min(1024, w - t0)
                            last = t0 + tw == w
                            ops = []
                            if last:
                                nd = tw - P
                                if nd > 0:
                                    ops.append((0, nd, "max"))
                                ops.append((nd, P, "add"))
                            else:
                                ops.append((0, tw, "max"))

                            def mk(t0=t0, tw=tw, ops=ops, half=half, i=i):
                                def th():
                                    s = spool.tile([P, 1024], F32, tag="s")
                                    for part in range(0, tw, 512):
                                        pw = min(512, tw - part)
                                        nc.tensor.matmul(
                                            s[:, ds(part, pw)],
                                            lhsT=qa[0:H, ts(i, P)],
                                            rhs=ka[0:H, ds(t0 + part, pw)],
                                            start=True,
                                            stop=True,
                                        )
                                    for off, wd, op in ops:
                                        st = steps[half]
                                        prev = (
                                            3.0e38
                                            if st == 0
                                            else negm[:, ds(half * 4 + st - 1, 1)]
                                        )
                                        in1 = (
                                            trqk_sb[:, :]
                                            if op == "add"
                                            else neginf[:, 0:wd]
                                        )
                                        nc.vector.tensor_tensor_reduce(
                                            out=scr[:, 0:wd],
                                            in0=s[:, ds(off, wd)],
                                            in1=in1,
                                            scale=-1.0,
                                            scalar=prev,
                                            op0=(
                                                mybir.AluOpType.add
                                                if op == "add"
                                                else mybir.AluOpType.max
                                            ),
                                            op1=mybir.AluOpType.min,
                                            accum_out=negm[
                                                :, ds(half * 4 + st, 1)
                                            ],
                                        )
                                        steps[half] = st + 1

                                return th

                            thunks.append(mk())
                    state[("negm", r)] = (negm, steps)
                    return thunks

                def emit_negm(r):
                    """-m row vector via matmul against identity + ACT copy
                    into qa row 64."""
                    negm, steps = state.pop(("negm", r))
                    rowm = rmpool.tile([1, 256], F32, tag="rm")
                    for half in range(2):
                        nc.tensor.matmul(
                            rowm[0:1, ds(half * P, P)],
                            lhsT=negm[:, ds(half * 4 + steps[half] - 1, 1)],
                            rhs=id_sb,
                            start=True,
                            stop=True,
                        )
                    nc.scalar.copy(
                        out=qa[H : H + 1, ds(2 * r * P, 256)], in_=rowm[0:1, :]
                    )

                def stp_thunks(r):
                    """S^T tiles for pair r (groups of 2 j), diag mask via
                    PE-accumulated constant, exp -> P^T fp16 in SBUF."""
                    i0, i1 = 2 * r, 2 * r + 1
                    pblk = ds(i0 * P, 256)
                    ptiles = []
                    thunks = []
                    j = 0
                    while j <= i1:
                        js = list(range(j, min(j + 2, i1 + 1)))
                        offs = []
                        off = 0
                        for jj in js:
                            wdt = P if jj == i1 else 256
                            offs.append((jj, off, wdt))
                            off += wdt
                        used = off
                        pts = ptpool.tile([P, 512], F16)
                        holder = [None]
                        ptiles.append((holder, pts, offs))

                        def mk(js=js, offs=offs, used=used, pts=pts,
                               holder=holder):
                            def th():
                                stp = stpool.tile([P, 512], F32, tag="stp")
                                holder[0] = stp
                                for (jj, off, wdt) in offs:
                                    diag = jj in (i0, i1)
                                    nc.tensor.matmul(
                                        stp[:, ds(off, wdt)],
                                        lhsT=ka[:, ts(jj, P)],
                                        rhs=(
                                            qa[:, ts(i1, P)]
                                            if wdt == P
                                            else qa[:, pblk]
                                        ),
                                        start=True,
                                        stop=not diag,
                                        skip_group_check=diag,
                                    )
                                    if jj == i0 or jj == i1:
                                        nc.tensor.matmul(
                                            stp[:, ds(off, P)],
                                            lhsT=id_sb,
                                            rhs=trkq_sb,
                                            start=False,
                                            stop=True,
                                            skip_group_check=True,
                                        )
                                nc.scalar.activation(
                                    out=pts[:, 0:used],
                                    in_=stp[:, 0:used],
                                    func=mybir.ActivationFunctionType.Exp,
                                    bias=0.0,
                                    scale=1.0,
                                )

                            return th

                        thunks.append(mk())
                        j = js[-1] + 1
                    state[("pts", r)] = ptiles
                    return thunks

                def av_thunks(r):
                    """Per-i AV accumulation [q,k]-form + normalize + DMA."""
                    i0, i1 = 2 * r, 2 * r + 1
                    ptiles = state.pop(("pts", r))
                    av = avpool.tile([P, 2, H + 1], F32, tag="av")
                    thunks = []
                    for half, ilim in ((0, i0), (1, i1)):
                        mms = []
                        for holder, pts, offs in ptiles:
                            for jj, off, wdt in offs:
                                if jj > ilim:
                                    continue
                                o = off if wdt == P else off + half * P
                                mms.append((jj, pts, o))
                        for gi in range(0, len(mms), 6):
                            grp = mms[gi : gi + 6]

                            def mk(grp=grp, half=half, ilim=ilim):
                                def th():
                                    for jj, pts, o in grp:
                                        nc.tensor.matmul(
                                            av[:, half, :],
                                            lhsT=pts[:, ds(o, P)],
                                            rhs=vt[:, jj, :],
                                            start=(jj == 0),
                                            stop=(jj == ilim),
                                        )

                                return th

                            thunks.append(mk())

                    def fin():
                        r2 = rlpool.tile([P, 2], F32)
                        nc.vector.reciprocal(r2, av[:, :, H])
                        osb = opool.tile([P, 2, H], F32)
                        for half in range(2):
                            nc.vector.tensor_scalar_mul(
                                osb[:, half, :],
                                av[:, half, 0:H],
                                r2[:, ds(half, 1)],
                            )
                        nc.sync.dma_start(
                            out=out[ds(2 * r * P, 2 * P), :].rearrange(
                                "(c p) h -> p c h", p=P
                            ),
                            in_=osb,
                        )

                    thunks.append(fin)
                    return thunks

                def emit_iter(it):
                    A = spass_thunks(it) if it < NPAIR else []
                    B = av_thunks(it - 2) if it >= 2 else []
                    if 1 <= it <= NPAIR:
                        emit_negm(it - 1)
                    C = stp_thunks(it - 1) if 1 <= it <= NPAIR else []
                    # PE-order merge: C throttled by ACT exp pace, so put
                    # A/B work between C groups; C[0:2] fill the stp bufs.
                    for th in C[0:2]:
                        th()
                    C = C[2:]
                    ab = A + B
                    ci = 0
                    ai = 0
                    while ci < len(C) or ai < len(ab):
                        if ci < len(C):
                            C[ci]()
                            ci += 1
                        for _ in range(2):
                            if ai < len(ab):
                                ab[ai]()
                                ai += 1
                        if ci >= len(C):
                            while ai < len(ab):
                                ab[ai]()
                                ai += 1

                for it in range(NPAIR + 2):
                    emit_iter(it)

                if DEBUG:
                    nc.sync.dma_start(out=qa_d[:, :], in_=qa)
                    nc.sync.dma_start(out=ka_d[:, :], in_=ka)
                    nc.sync.dma_start(
                        out=vt_d[:, :],
                        in_=vt.rearrange("p t h -> p (t h)"),
                    )

    nc.compile()
    return nc


def _host_prep(input, Wq, bq, Wk, bk, Wv, bv):
    input = np.asarray(input, dtype=np.float32)
    Wq = np.asarray(Wq, dtype=np.float32)
    Wk = np.asarray(Wk, dtype=np.float32)
    Wv = np.asarray(Wv, dtype=np.float32)
    bq = np.asarray(bq, dtype=np.float32)
    bk = np.asarray(bk, dtype=np.float32)
    bv = np.asarray(bv, dtype=np.float32)
    scale = np.float32(np.sqrt(np.float32(H)))

    wqkT = np.ascontiguousarray(
        np.concatenate([Wq * scale, Wk], axis=0).T
    ).astype(np.float16)
    wvT = np.ascontiguousarray(Wv.T).astype(np.float16)
    bqk = np.concatenate([bq * scale, bk]).reshape(P, 1).astype(np.float32)
    bv4 = np.tile(bv.reshape(1, H), (1, 4)).astype(np.float32)
    NEG = np.float32(-1.0e9)
    qq, kk = np.indices((P, P))
    trqk = np.where(kk > qq, NEG, np.float32(0))  # [q, k] causal mask
    # [k, q] mask, fp16 (PE-accumulated into S^T): -60000 >> score range
    trkq = np.ascontiguousarray(
        np.where(kk > qq, np.float16(-60000), np.float16(0)).T
    )
    id128 = np.eye(P, dtype=np.float16)

    shared = {
        "wqkT": wqkT,
        "wvT": wvT,
        "bqk": bqk,
        "bv4": bv4,
        "trqk": trqk,
        "trkq": trkq,
        "id128": id128,
    }
    in_maps = []
    for b in range(B):
        m = dict(shared)
        m["x16t"] = np.ascontiguousarray(input[b].T).astype(np.float16)
        in_maps.append(m)
    return in_maps


def kernel(input, Wq, bq, Wk, bk, Wv, bv, mask=None, **_ignored):
    # mask is all-False by construction (spec fill: zeros) -> identity.
    from concourse.bass_utils import run_bass_kernel_spmd

    if "nc" not in _CACHE:
        _CACHE["nc"] = build_nc()
    nc = _CACHE["nc"]
    in_maps = _host_prep(input, Wq, bq, Wk, bk, Wv, bv)
    res = run_bass_kernel_spmd(nc, in_maps, core_ids=list(range(B)))
    return np.stack([res.results[b]["out"] for b in range(B)], axis=0)


# revision 34
# speedup vs baseline: 1.2797x; 1.2797x over previous
"""Single-head causal attention (B=8, T=2048, E=1024, H=64) on 8 TRN2 cores.

Sharding: data-parallel over batch B - one batch element per NeuronCore;
projection weights replicated. Per-core kernel:

  q = x @ Wq.T + bq ; k = x @ Wk.T + bk ; v = x @ Wv.T + bv
  s = (q @ k.T) * sqrt(H)  (scale folded into Wq/bq on host)
  causal softmax(s) @ v

Design (v2) - all-matmul PE stream, no PE transposes (keeps HAM warm):
  - x pre-transposed + cast fp16 on host -> contiguous DMA (no xbar).
  - Q^T/K^T packed projection (full 128-wide array), written into
    65-partition tiles qa/ka; ka row 64 = ones, qa row 64 = -m_i (per
    q-block row max, folded into the scores via the augmented
    contraction: S^T = ka.T-slice @ qa-slice includes -m per column).
  - Max pass: S = q-block @ k^T chunks in PSUM, row max via fused
    tensor_tensor_reduce (2 elem/cycle, negation folded via scale=-1 +
    min-reduce), chained across chunks -> -m in fp16.
  - -m row vector obtained with a tiny matmul against identity
    (negm^T = negm.T @ I) - no transpose-mode ops anywhere.
  - S^T computed directly by a second matmul (lhsT = ka j-block,
    rhs = qa pair-block), exp on ACT reads PSUM -> writes P^T (bf16)
    straight to SBUF. Diagonal/causal masking via one fp16 multiply.
  - q-blocks processed in pairs (256-wide moving operand).
  - AV in O^T form: lhsT = v-tile [128,65] (col 64 = ones -> row sum l),
    rhs = P^T pair tile, accumulate O^T [65, 256] in PSUM. Normalize by
    broadcasting 1/l (DMA broadcast) + one multiply; output stored as
    O^T [64, T] and un-transposed on host.
"""

import sys

sys.path.insert(0, "/opt/trn_rl_repo")

import numpy as np

import concourse.bass as bass
import concourse.mybir as mybir
from concourse import bacc
from concourse.bass import ds, ts
from concourse.tile import TileContext

B, T, E, H = 8, 2048, 1024, 64
P = 128
NE = E // P  # 8 e-chunks
NT = T // P  # 16 t-tiles
NPAIR = NT // 2  # 8 q-block pairs
F16 = mybir.dt.float16
BF16 = mybir.dt.bfloat16
F32 = mybir.dt.float32

_CACHE = {}
DEBUG = False


def build_nc():
    nc = bacc.Bacc("TRN2", num_devices=8)
    x16t = nc.declare_dram_parameter("x16t", [E, T], F16, isOutput=False)
    wqkT = nc.declare_dram_parameter("wqkT", [E, P], F16, isOutput=False)
    wvT = nc.declare_dram_parameter("wvT", [E, H], F16, isOutput=False)
    bqk = nc.declare_dram_parameter("bqk", [P, 1], F32, isOutput=False)
    bv4 = nc.declare_dram_parameter("bv4", [1, 4 * H], F32, isOutput=False)
    trqk = nc.declare_dram_parameter("trqk", [P, P], F32, isOutput=False)
    trkq = nc.declare_dram_parameter("trkq", [P, P], F16, isOutput=False)
    id128 = nc.declare_dram_parameter("id128", [P, P], F16, isOutput=False)
    out = nc.declare_dram_parameter("out", [T, H], F32, isOutput=True)
    if DEBUG:
        qa_d = nc.declare_dram_parameter("qa_d", [H + 1, T], F16, isOutput=True)
        ka_d = nc.declare_dram_parameter("ka_d", [H + 1, T], F16, isOutput=True)
        vt_d = nc.declare_dram_parameter(
            "vt_d", [P, NT * (H + 1)], F16, isOutput=True
        )

    with TileContext(nc) as tc:
        with (
            tc.tile_pool(name="const", bufs=1) as cpool,
            tc.tile_pool(name="xt", bufs=1) as xtpool,
            tc.tile_pool(name="qk", bufs=1) as qkpool,
            tc.tile_pool(name="vp", bufs=1) as vpool,
            tc.tile_pool(name="pt", bufs=18) as ptpool,
            tc.tile_pool(name="negm", bufs=2) as negmpool,
            tc.tile_pool(name="rl", bufs=2) as rlpool,
            tc.tile_pool(name="osb", bufs=2) as opool,
        ):
            # ---- constants ----
            wqk_sb = cpool.tile([P, NE, P], F16)
            nc.sync.dma_start(
                out=wqk_sb, in_=wqkT.rearrange("(c p) h -> p c h", p=P)
            )
            wv_sb = cpool.tile([P, NE, H], F16)
            nc.sync.dma_start(out=wv_sb, in_=wvT.rearrange("(c p) h -> p c h", p=P))
            bqk_sb = cpool.tile([P, 1], F32)
            nc.sync.dma_start(out=bqk_sb, in_=bqk[:, :])
            bv_sb = cpool.tile([P, 4 * H], F32)
            nc.sync.dma_start(out=bv_sb, in_=bv4[:, :].to_broadcast((P, 4 * H)))
            trqk_sb = cpool.tile([P, P], F32)
            nc.sync.dma_start(out=trqk_sb, in_=trqk[:, :])
            trkq_sb = cpool.tile([P, P], F16)
            nc.sync.dma_start(out=trkq_sb, in_=trkq[:, :])
            id_sb = cpool.tile([P, P], F16)
            nc.sync.dma_start(out=id_sb, in_=id128[:, :])

            # ---- x^T chunks (contiguous, pre-transposed on host) ----
            xt = xtpool.tile([P, NE, T], F16)
            for c in range(NE):
                nc.sync.dma_start(out=xt[:, c, :], in_=x16t[ts(c, P), :])

            # qa/ka: rows 0..63 = q^T/k^T; row 64: ka = ones, qa = -m
            qa = qkpool.tile([H + 1, T], F16)
            ka = qkpool.tile([H + 1, T], F16)
            nc.vector.memset(ka[H : H + 1, :], 1.0)

            # vt: [128, NT, H+1]; col H = ones (row-sum trick)
            vt = vpool.tile([P, NT, H + 1], F16)
            nc.vector.memset(vt, 1.0)
            neginf = cpool.tile([P, 1024], F32)
            nc.vector.memset(neginf, -3.0e38)

            # ---- phase 1: projections (DMA-overlapped c-major loop) ----
            with (
                tc.tile_pool(name="accq", bufs=1, space="PSUM") as accqp,
                tc.tile_pool(name="accv", bufs=1, space="PSUM") as accvp,
            ):
                acc0 = accqp.tile([P, 1024], F32, tag="a0")
                acc1 = accqp.tile([P, 1024], F32, tag="a1")
                vacc = accvp.tile([P, NT * H], F32, tag="v")
                accs = [acc0, acc1]
                for c in range(NE):
                    for n in range(2):
                        for h in range(2):
                            nc.tensor.matmul(
                                accs[n][:, ds(h * 512, 512)],
                                lhsT=wqk_sb[:, c, :],
                                rhs=xt[:, c, ds(n * 1024 + h * 512, 512)],
                                start=(c == 0),
                                stop=(c == NE - 1),
                            )
                    for t in range(NT):
                        # start=True clears the whole PSUM bank's has_written
                        # bits, so only the first t-group per bank may set it;
                        # later groups overwrite via has_written=0.
                        nc.tensor.matmul(
                            vacc[:, ds(t * H, H)],
                            lhsT=xt[:, c, ts(t, P)],
                            rhs=wv_sb[:, c, :],
                            start=(c == 0 and t % 8 == 0),
                            stop=(c == NE - 1),
                            skip_group_check=True,
                        )
                # write q^T/k^T (+bias) into qa/ka rows 0..63
                for n in range(2):
                    nc.scalar.activation(
                        out=qa[0:H, ds(n * 1024, 1024)],
                        in_=accs[n][0:H, :],
                        func=mybir.ActivationFunctionType.Identity,
                        bias=bqk_sb[0:H, :],
                        scale=1.0,
                    )
                    nc.scalar.activation(
                        out=ka[0:H, ds(n * 1024, 1024)],
                        in_=accs[n][H:P, :],
                        func=mybir.ActivationFunctionType.Identity,
                        bias=bqk_sb[H:P, :],
                        scale=1.0,
                    )
                # v + bias -> vt cols 0..63 (quads of t-tiles)
                for g in range(4):
                    nc.vector.tensor_add(
                        vt[:, ds(g * 4, 4), 0:H],
                        vacc[:, ds(g * 4 * H, 4 * H)].rearrange(
                            "p (t h) -> p t h", t=4
                        ),
                        bv_sb[:, :].rearrange("p (t h) -> p t h", t=4),
                    )

            # ---- phase 2: attention over q-block pairs ----
            with (
                tc.tile_pool(name="sps", bufs=2, space="PSUM") as spool,
                tc.tile_pool(name="stp", bufs=2, space="PSUM") as stpool,
                tc.tile_pool(name="av", bufs=1, space="PSUM") as avpool,
                tc.tile_pool(name="rowm", bufs=1, space="PSUM") as rmpool,
            ):
                scr = cpool.tile([P, 1024], F32)  # TT-reduce throwaway out
                state = {}

                def spass_thunks(r):
                    """Max pass for pair r: S chunks, causal diag add,
                    two-level reduce_max -> negm fp16 [128,1] per i."""
                    negm = negmpool.tile([P, 2], F16)
                    mx = negmpool.tile([P, 2, 2], F32, tag="mx")
                    thunks = []
                    for half in range(2):
                        i = 2 * r + half
                        w = (i + 1) * P
                        ntl = (w + 1023) // 1024
                        for tix, t0 in enumerate(range(0, w, 1024)):
                            tw = min(1024, w - t0)
                            last = t0 + tw == w

                            def mk(t0=t0, tw=tw, last=last, tix=tix,
                                   half=half, i=i, ntl=ntl):
                                def th():
                                    s = spool.tile([P, 1024], F32, tag="s")
                                    for part in range(0, tw, 512):
                                        pw = min(512, tw - part)
                                        nc.tensor.matmul(
                                            s[:, ds(part, pw)],
                                            lhsT=qa[0:H, ts(i, P)],
                                            rhs=ka[0:H, ds(t0 + part, pw)],
                                            start=True,
                                            stop=True,
                                        )
                                    if last:
                                        nc.vector.tensor_add(
                                            s[:, ds(tw - P, P)],
                                            s[:, ds(tw - P, P)],
                                            trqk_sb[:, :],
                                        )
                                    if ntl == 1:
                                        nc.vector.reduce_max(
                                            out=negm[:, ds(half, 1)],
                                            in_=s[:, 0:tw],
                                            axis=mybir.AxisListType.X,
                                            negate=True,
                                        )
                                    else:
                                        nc.vector.reduce_max(
                                            out=mx[:, half, ds(tix, 1)],
                                            in_=s[:, 0:tw],
                                            axis=mybir.AxisListType.X,
                                        )
                                        if last:
                                            nc.vector.reduce_max(
                                                out=negm[:, ds(half, 1)],
                                                in_=mx[:, half, :],
                                                axis=mybir.AxisListType.X,
                                                negate=True,
                                            )

                                return th

                            thunks.append(mk())
                    state[("negm", r)] = negm
                    return thunks

                def emit_negm(r):
                    """-m row vector via matmul against identity + ACT copy
                    into qa row 64."""
                    negm = state.pop(("negm", r))
                    rowm = rmpool.tile([1, 256], F32, tag="rm")
                    for half in range(2):
                        nc.tensor.matmul(
                            rowm[0:1, ds(half * P, P)],
                            lhsT=negm[:, ds(half, 1)],
                            rhs=id_sb,
                            start=True,
                            stop=True,
                        )
                    nc.scalar.copy(
                        out=qa[H : H + 1, ds(2 * r * P, 256)], in_=rowm[0:1, :]
                    )

                def stp_thunks(r):
                    """S^T tiles for pair r (groups of 2 j), diag mask via
                    PE-accumulated constant, exp -> P^T fp16 in SBUF."""
                    i0, i1 = 2 * r, 2 * r + 1
                    pblk = ds(i0 * P, 256)
                    ptiles = []
                    thunks = []
                    j = 0
                    while j <= i1:
                        js = list(range(j, min(j + 2, i1 + 1)))
                        offs = []
                        off = 0
                        for jj in js:
                            wdt = P if jj == i1 else 256
                            offs.append((jj, off, wdt))
                            off += wdt
                        used = off
                        pts = ptpool.tile([P, 512], F16)
                        holder = [None]
                        ptiles.append((holder, pts, offs))

                        def mk(js=js, offs=offs, used=used, pts=pts,
                               holder=holder):
                            def th():
                                stp = stpool.tile([P, 512], F32, tag="stp")
                                holder[0] = stp
                                for (jj, off, wdt) in offs:
                                    diag = jj in (i0, i1)
                                    nc.tensor.matmul(
                                        stp[:, ds(off, wdt)],
                                        lhsT=ka[:, ts(jj, P)],
                                        rhs=(
                                            qa[:, ts(i1, P)]
                                            if wdt == P
                                            else qa[:, pblk]
                                        ),
                                        start=True,
                                        stop=not diag,
                                        skip_group_check=diag,
                                    )
                                    if jj == i0 or jj == i1:
                                        nc.tensor.matmul(
                                            stp[:, ds(off, P)],
                                            lhsT=id_sb,
                                            rhs=trkq_sb,
                                            start=False,
                                            stop=True,
                                            skip_group_check=True,
                                        )
                                nc.scalar.activation(
                                    out=pts[:, 0:used],
                                    in_=stp[:, 0:used],
                                    func=mybir.ActivationFunctionType.Exp,
                                    bias=0.0,
                                    scale=1.0,
                                )

                            return th

                        thunks.append(mk())
                        j = js[-1] + 1
                    state[("pts", r)] = ptiles
                    return thunks

                def av_thunks(r):
                    """Per-i AV accumulation [q,k]-form + normalize + DMA."""
                    i0, i1 = 2 * r, 2 * r + 1
                    ptiles = state.pop(("pts", r))
                    av = avpool.tile([P, 2, H + 1], F32, tag="av")
                    thunks = []
                    for half, ilim in ((0, i0), (1, i1)):
                        mms = []
                        for holder, pts, offs in ptiles:
                            for jj, off, wdt in offs:
                                if jj > ilim:
                                    continue
                                o = off if wdt == P else off + half * P
                                mms.append((jj, pts, o))
                        for gi in range(0, len(mms), 6):
                            grp = mms[gi : gi + 6]

                            def mk(grp=grp, half=half, ilim=ilim):
                                def th():
                                    for jj, pts, o in grp:
                                        nc.tensor.matmul(
                                            av[:, half, :],
                                            lhsT=pts[:, ds(o, P)],
                                            rhs=vt[:, jj, :],
                                            start=(jj == 0),
                                            stop=(jj == ilim),
                                        )

                                return th

                            thunks.append(mk())

                    def fin():
                        r2 = rlpool.tile([P, 2], F32)
                        nc.vector.reciprocal(r2, av[:, :, H])
                        osb = opool.tile([P, 2, H], F32)
                        for half in range(2):
                            nc.vector.tensor_scalar_mul(
                                osb[:, half, :],
                                av[:, half, 0:H],
                                r2[:, ds(half, 1)],
                            )
                        nc.sync.dma_start(
                            out=out[ds(2 * r * P, 2 * P), :].rearrange(
                                "(c p) h -> p c h", p=P
                            ),
                            in_=osb,
                        )

                    thunks.append(fin)
                    return thunks

                def emit_iter(it):
                    A = spass_thunks(it) if it < NPAIR else []
                    B = av_thunks(it - 2) if it >= 2 else []
                    if 1 <= it <= NPAIR:
                        emit_negm(it - 1)
                    C = stp_thunks(it - 1) if 1 <= it <= NPAIR else []
                    # PE-order merge: C throttled by ACT exp pace, so put
                    # A/B work between C groups; C[0:2] fill the stp bufs.
                    for th in C[0:2]:
                        th()
                    C = C[2:]
                    ab = A + B
                    ci = 0
                    ai = 0
                    while ci < len(C) or ai < len(ab):
                        if ci < len(C):
                            C[ci]()
                            ci += 1
                        for _ in range(2):
                            if ai < len(ab):
                                ab[ai]()
                                ai += 1
                        if ci >= len(C):
                            while ai < len(ab):
                                ab[ai]()
                                ai += 1

                for it in range(NPAIR + 2):
                    emit_iter(it)

                if DEBUG:
                    nc.sync.dma_start(out=qa_d[:, :], in_=qa)
                    nc.sync.dma_start(out=ka_d[:, :], in_=ka)
                    nc.sync.dma_start(
                        out=vt_d[:, :],
                        in_=vt.rearrange("p t h -> p (t h)"),
                    )

    nc.compile()
    return nc


def _host_prep(input, Wq, bq, Wk, bk, Wv, bv):
    input = np.asarray(input, dtype=np.float32)
    Wq = np.asarray(Wq, dtype=np.float32)
    Wk = np.asarray(Wk, dtype=np.float32)
    Wv = np.asarray(Wv, dtype=np.float32)
    bq = np.asarray(bq, dtype=np.float32)
    bk = np.asarray(bk, dtype=np.float32)
    bv = np.asarray(bv, dtype=np.float32)
    scale = np.float32(np.sqrt(np.float32(H)))

    wqkT = np.ascontiguousarray(
        np.concatenate([Wq * scale, Wk], axis=0).T
    ).astype(np.float16)
    wvT = np.ascontiguousarray(Wv.T).astype(np.float16)
    bqk = np.concatenate([bq * scale, bk]).reshape(P, 1).astype(np.float32)
    bv4 = np.tile(bv.reshape(1, H), (1, 4)).astype(np.float32)
    NEG = np.float32(-1.0e9)
    qq, kk = np.indices((P, P))
    trqk = np.where(kk > qq, NEG, np.float32(0))  # [q, k] causal mask
    # [k, q] mask, fp16 (PE-accumulated into S^T): -60000 >> score range
    trkq = np.ascontiguousarray(
        np.where(kk > qq, np.float16(-60000), np.float16(0)).T
    )
    id128 = np.eye(P, dtype=np.float16)

    shared = {
        "wqkT": wqkT,
        "wvT": wvT,
        "bqk": bqk,
        "bv4": bv4,
        "trqk": trqk,
        "trkq": trkq,
        "id128": id128,
    }
    in_maps = []
    for b in range(B):
        m = dict(shared)
        m["x16t"] = np.ascontiguousarray(input[b].T).astype(np.float16)
        in_maps.append(m)
    return in_maps


def kernel(input, Wq, bq, Wk, bk, Wv, bv, mask=None, **_ignored):
    # mask is all-False by construction (spec fill: zeros) -> identity.
    from concourse.bass_utils import run_bass_kernel_spmd

    if "nc" not in _CACHE:
        _CACHE["nc"] = build_nc()
    nc = _CACHE["nc"]
    in_maps = _host_prep(input, Wq, bq, Wk, bk, Wv, bv)
    res = run_bass_kernel_spmd(nc, in_maps, core_ids=list(range(B)))
    return np.stack([res.results[b]["out"] for b in range(B)], axis=0)
